# revision 63
# baseline (speedup 1.0000x reference)
"""Multi-head softmax attention (b=4, s=2048, d=1024, 16 heads) on 8 trn2 cores.

Sharding: 2D over (batch, head-half). Core c handles batch c//2, heads
[8*(c%2), 8*(c%2)+8). Each core computes its QKV projections, attention for
its 8 heads, and a partial output projection (row-parallel over its 512
attn-out columns). Host sums the two partials per batch.

Schedule: the kernel is ACT-bound (256 exp activations of [128,1024] at
~1.1us each). Emission is software-pipelined around that stream: scores for
group g+2 are emitted at group g, the exp for group g consumes them, and
attnV lags behind on a deep pp pool. All projection work (V/QK/out) is
chopped into accumulation chains dripped into per-group PE slack by
deadline, so the exp stream starts ~10us in and never starves.
"""

import numpy as np

HIDDEN = 1024
SEQ = 2048
BATCH = 4
HEADS = 16
HG = 8  # heads per core
HD = 64  # head dim

_CACHE = {}
_TRACE = False  # test.py sets this for profiling runs
_DEGEN = False  # debug: baseline-like schedule (no deferral, eager chains)
LAST_RESULT = None

# host-side column-block order of the packed qkv weight: K/Q of pair 0 first
# so the first score groups only need a 256-column DMA slice.
CCX_ORDER = [4, 0, 5, 1, 6, 2, 7, 3]
CCX_OFF = {ccx: i * 128 for i, ccx in enumerate(CCX_ORDER)}


def _build_nc():
    from collections import deque

    import concourse.mybir as mybir
    import concourse.tile as tile
    from concourse import bacc

    f32 = mybir.dt.float32
    f16 = mybir.dt.float16
    Exp = mybir.ActivationFunctionType.Exp

    nc = bacc.Bacc("TRN2", target_bir_lowering=False, debug=False)
    xT = nc.dram_tensor("xT", [128, 8, SEQ], f16, kind="ExternalInput").ap()
    wqk = nc.dram_tensor("wqk", [128, 8, 1024], f16, kind="ExternalInput").ap()
    wv = nc.dram_tensor("wv", [128, 8, 512], f16, kind="ExternalInput").ap()
    wout = nc.dram_tensor("wout", [128, 4, HIDDEN], f16, kind="ExternalInput").ap()
    bqk = nc.dram_tensor("bqk", [128, 8], f32, kind="ExternalInput").ap()
    outp = nc.dram_tensor("outp", [SEQ, HIDDEN], f16, kind="ExternalOutput").ap()

    with tile.TileContext(nc) as tc:
        with (
            tc.tile_pool(name="persist", bufs=1) as pers,
            tc.tile_pool(name="pp", bufs=14) as pppool,
            tc.tile_pool(name="small", bufs=2) as small,
            tc.tile_pool(name="obuf", bufs=3) as obuf,
            tc.tile_pool(name="ps_sc", bufs=2, space="PSUM") as ps_sc,
            tc.tile_pool(name="ps_ac", bufs=1, space="PSUM") as ps_ac,
            tc.tile_pool(name="ps_aux", bufs=2, space="PSUM") as ps_aux,
        ):
            xt16 = pers.tile([128, 8, SEQ], f16, tag="xt16", name="xt16")
            wqk16 = pers.tile([128, 8, 1024], f16, tag="wqk16", name="wqk16")
            wv16 = pers.tile([128, 8, 512], f16, tag="wv16", name="wv16")
            qk = [pers.tile([128, SEQ], f16, tag=f"qk{i}", name=f"qk{i}") for i in range(8)]
            vt = [pers.tile([128, HG, 128], f16, tag=f"vt{i}", name=f"vt{i}") for i in range(16)]
            aot = [pers.tile([128, SEQ], f16, tag=f"aot{i}", name=f"aot{i}") for i in range(4)]
            wout_sb = pers.tile([128, 4, HIDDEN], f16, tag="wo", name="wo")
            bqk_sb = pers.tile([128, 8], f32, tag="bqk", name="bqk")
            ones8 = pers.tile([128, HG], f16, tag="ones8", name="ones8")

            # ---------------- DMA emission, need-ordered ----------------
            nc.sync.dma_start(bqk_sb[:], bqk[:])
            for h2 in range(0, 8, 2):  # K0+Q0 slices (host packs them first);
                # 2 hc per transfer: fewer Sync issues, keeps queue parallelism
                nc.sync.dma_start(
                    wqk16[:, h2 : h2 + 2, 0:256], wqk[:, h2 : h2 + 2, 0:256]
                )
            for h2 in range(0, 8, 2):  # xt token-quarter 0
                nc.sync.dma_start(
                    xt16[:, h2 : h2 + 2, 0:512], xT[:, h2 : h2 + 2, 0:512]
                )
            for h2 in range(0, 8, 2):
                nc.sync.dma_start(wv16[:, h2 : h2 + 2, :], wv[:, h2 : h2 + 2, :])
            for t in range(1, 4):
                for h2 in range(0, 8, 2):
                    nc.sync.dma_start(
                        xt16[:, h2 : h2 + 2, t * 512 : (t + 1) * 512],
                        xT[:, h2 : h2 + 2, t * 512 : (t + 1) * 512],
                    )
            for h2 in range(0, 8, 2):  # rest of qkv weights
                nc.sync.dma_start(
                    wqk16[:, h2 : h2 + 2, 256:1024], wqk[:, h2 : h2 + 2, 256:1024]
                )
            for i in range(0, 4, 2):
                nc.sync.dma_start(
                    wout_sb[:, i : i + 2, :], wout[:, i : i + 2, :]
                )
            nc.vector.memset(ones8[:], 1.0)
            # warm the ACT exp table during the DMA lead (table load ~2.7us)
            warm = pers.tile([128, HG], f32, tag="warm", name="warm")
            nc.scalar.activation(warm[:], ones8[:], Exp)
            for t in range(16):
                nc.vector.memset(vt[t][:, :, HD + 1 : 128], 0.0)
                nc.vector.tensor_copy(vt[t][:, :, HD], ones8[:])

            # ---------------- chain builders ----------------
            def aux_psum():
                return ps_aux.tile([128, 512], f32, tag="aux", name="aux")

            qk_done = set()  # (tt, ccx) fully emitted
            vt_done = [False] * 16

            def qk_chain(tt, ccx):
                """Q/K projection chain: qk[ccx][:, tt*512:+512] = W_ccx^T x."""
                cell = {}
                off = CCX_OFF[ccx]
                ops = []

                def mk_mm(hc):
                    def f():
                        if "ps" not in cell:
                            cell["ps"] = aux_psum()
                        nc.tensor.matmul(
                            cell["ps"][:],
                            wqk16[:, hc, off : off + 128],
                            xt16[:, hc, tt * 512 : (tt + 1) * 512],
                            start=(hc == 0),
                            stop=(hc == 7),
                        )
                    return f

                for hc in range(8):
                    ops.append((240, mk_mm(hc)))

                def ev():
                    nc.vector.tensor_scalar_add(
                        qk[ccx][:, tt * 512 : (tt + 1) * 512],
                        cell["ps"][:],
                        bqk_sb[:, ccx : ccx + 1],
                    )
                    qk_done.add((tt, ccx))
                return ops + [(0, ev)]

            def v_chain(t):
                """V projection chain for token chunk t (token-major + ones col)."""
                cell = {}
                ops = []

                def mk_mm(hc):
                    def f():
                        if "ps" not in cell:
                            cell["ps"] = aux_psum()
                        nc.tensor.matmul(
                            cell["ps"][:],
                            xt16[:, hc, t * 128 : (t + 1) * 128],
                            wv16[:, hc, :],
                            start=(hc == 0),
                            stop=(hc == 7),
                        )
                    return f

                for hc in range(8):
                    ops.append((240, mk_mm(hc)))

                def ev():
                    nc.vector.tensor_copy(
                        vt[t][:, :, 0:HD],
                        cell["ps"][:].rearrange("p (h d) -> p h d", h=HG),
                    )
                    vt_done[t] = True
                return ops + [(0, ev)]

            def out_chain(tch, nt):
                """Output projection chain for (token chunk, 512-col half)."""
                cell = {}
                ops = []

                def mk_mm(pair_):
                    def f():
                        if "ps" not in cell:
                            cell["ps"] = aux_psum()
                        nc.tensor.matmul(
                            cell["ps"][:],
                            aot[pair_][:, tch * 128 : (tch + 1) * 128],
                            wout_sb[:, pair_, nt * 512 : (nt + 1) * 512],
                            start=(pair_ == 0),
                            stop=(pair_ == 3),
                        )
                    return f

                for pair_ in range(4):
                    ops.append((240, mk_mm(pair_)))

                def ev():
                    ot = obuf.tile([128, 512], f16, tag="ot", name="ot")
                    nc.vector.tensor_copy(ot[:], cell["ps"][:])
                    nc.sync.dma_start(
                        outp[tch * 128 : (tch + 1) * 128, nt * 512 : (nt + 1) * 512],
                        ot[:],
                    )
                return ops + [(0, ev)]

            # ---------------- group primitives ----------------
            # group g: pair = g>>6, qt = (g>>4)&3, kc = g&15
            sc_of = {}
            pp_of = {}
            acc_cur = {}

            def emit_S(g):
                pair, qt, kc = g >> 6, (g >> 4) & 3, g & 15
                assert (kc // 4, 4 + pair) in qk_done, f"K chain missing for g={g}"
                assert (qt, pair) in qk_done, f"Q chain missing for g={g}"
                qtile = qk[pair]
                ktile = qk[4 + pair]
                sc = ps_sc.tile([128, 1024], f32, tag="sc", name="sc")
                sc_of[g] = sc
                # standalone LDWEIGHTS: the PE queue can hoist it over the
                # in-flight full-config matmuls, hiding the ~107ns load that a
                # self-loading tiled matmul pays serially at the mode switch
                nc.tensor.ldweights(
                    ktile[0:64, kc * 128 : (kc + 1) * 128],
                    tile_position=(0, 0),
                )
                nc.tensor.matmul(
                    sc[:, 0:512],
                    ktile[0:64, kc * 128 : (kc + 1) * 128],
                    qtile[0:64, qt * 512 : (qt + 1) * 512],
                    start=True,
                    stop=True,
                    tile_position=(0, 0),
                )
                nc.tensor.matmul(
                    sc[:, 512:1024],
                    ktile[64:128, kc * 128 : (kc + 1) * 128],
                    qtile[64:128, qt * 512 : (qt + 1) * 512],
                    start=True,
                    stop=True,
                    tile_position=(64, 0),
                )

            def emit_exp(g):
                pp = pppool.tile([128, 1024], f16, tag="pp", name="pp")
                pp_of[g] = pp
                nc.scalar.activation(pp[:], sc_of.pop(g)[:], Exp)

            def emit_aV(g):
                pair, qt, kc = g >> 6, (g >> 4) & 3, g & 15
                assert vt_done[kc], f"vt missing for g={g}"
                if kc == 0:
                    acc_cur["A"] = ps_ac.tile([128, 512], f32, tag="accA", name="accA")
                    acc_cur["B"] = ps_ac.tile([128, 512], f32, tag="accB", name="accB")
                pp = pp_of.pop(g)
                nc.tensor.matmul(
                    acc_cur["A"][:],
                    vt[kc][:, 2 * pair, :],
                    pp[:, 0:512],
                    start=(kc == 0),
                    stop=(kc == 15),
                )
                nc.tensor.matmul(
                    acc_cur["B"][:],
                    vt[kc][:, 2 * pair + 1, :],
                    pp[:, 512:1024],
                    start=(kc == 0),
                    stop=(kc == 15),
                )

            def emit_drain_norm(pair, qt):
                # drain PSUM accumulators promptly so the banks free fast
                parts = []
                for key, row0 in (("A", 0), ("B", 64)):
                    acc = acc_cur[key]
                    dn = small.tile([1, 512], f32, tag="dn", name="dn")
                    nc.vector.tensor_copy(dn[:], acc[64:65, :])
                    numer = small.tile([64, 512], f32, tag="numer", name="numer")
                    nc.vector.tensor_copy(numer[:], acc[0:64, :])
                    parts.append((dn, numer, row0))
                for dn, numer, row0 in parts:
                    bc = small.tile([64, 512], f32, tag="bc", name="bc")
                    nc.gpsimd.partition_broadcast(bc[:], dn[:])
                    rc = small.tile([64, 512], f32, tag="rc", name="rc")
                    scr = small.tile([64, 512], f32, tag="scr", name="scr")
                    nc.vector.reciprocal_approx_accurate(rc[:], bc[:], scr[:])
                    nc.vector.tensor_mul(
                        aot[pair][row0 : row0 + 64, qt * 512 : (qt + 1) * 512],
                        numer[:],
                        rc[:],
                    )

            # ---------------- lead ----------------
            lead_k = qk_chain(0, 4)
            lead_q = qk_chain(0, 0)
            for _, f in lead_k:
                f()
            for _, f in lead_q:
                f()
            emit_S(0)
            emit_S(1)

            # ---------------- chain queue with deadlines ----------------
            chainq = []  # sorted by due: [due, deque(ops), label]
            for t in range(16):  # V projection upfront: a clean serial lead
                for _, f in v_chain(t):
                    f()
            for tt in range(1, 4):
                chainq.append([4 * tt - 3, deque(qk_chain(tt, 4)), f"K0t{tt}"])
            for tt in range(1, 4):
                chainq.append([16 * tt - 3, deque(qk_chain(tt, 0)), f"Q0t{tt}"])
            for p in range(1, 4):
                for tt in range(4):
                    chainq.append(
                        [64 * p + 4 * tt - 7, deque(qk_chain(tt, 4 + p)), f"K{p}t{tt}"]
                    )
                for tt in range(4):
                    chainq.append(
                        [64 * p + 16 * tt - 9, deque(qk_chain(tt, p)), f"Q{p}t{tt}"]
                    )
            chainq.sort(key=lambda c: c[0])

            # ---------------- main group loop ----------------
            cursor = 0  # next aV unit to emit
            partial = []  # front chain being dripped; never preempted

            def fill_ops(budget):
                # emit chain ops one at a time; a partially-emitted chain owns
                # an aux-psum slot, so it must finish before any other starts
                while True:
                    if not partial:
                        if not chainq:
                            return budget
                        if budget <= 150:
                            return budget
                        partial.append(chainq.pop(0))
                    c = partial[0]
                    while c[1] and budget > 150:
                        cost, f = c[1].popleft()
                        f()
                        budget -= max(cost, 60)
                    if c[1]:
                        return budget
                    partial.pop()

            def force_finish_due(g):
                # complete the partial chain, then any chain whose due passed
                while partial and partial[0][0] <= g:
                    c = partial.pop()
                    while c[1]:
                        c[1].popleft()[1]()
                while chainq and chainq[0][0] <= g:
                    if partial:  # a later-due chain is mid-flight: finish it
                        c = partial.pop()
                    else:
                        c = chainq.pop(0)
                    while c[1]:
                        c[1].popleft()[1]()

            def caps(g):
                if _DEGEN:
                    return 10**9
                if g < 48:
                    return 1250
                if g >= 192:
                    return 1500
                return 1160

            for g in range(256):
                emit_exp(g)
                # S goes first: it is the only PE work on the exp stream's
                # critical path (exp(g+2) needs it); chain dues guarantee its
                # qk inputs were emitted in earlier groups
                if g + 2 < 256:
                    emit_S(g + 2)
                force_finish_due(g)
                budget = caps(g) - 330
                while cursor <= g and vt_done[cursor & 15] and (
                    budget > 0 or (g - cursor) >= 11
                ):
                    emit_aV(cursor)
                    budget -= 440
                    if (cursor & 15) == 15:
                        emit_drain_norm(cursor >> 6, (cursor >> 4) & 3)
                        p3, q3 = cursor >> 6, (cursor >> 4) & 3
                        if p3 == 3:  # out-proj cells become available
                            for i, (t4, nt) in enumerate(
                                (t4, nt) for t4 in range(4) for nt in range(2)
                            ):
                                chainq.append(
                                    [g + 10 + 2 * i, deque(out_chain(q3 * 4 + t4, nt)),
                                     f"O{q3}{t4}{nt}"]
                                )
                            chainq.sort(key=lambda c: c[0])
                    cursor += 1
                fill_ops(budget)

            # ---------------- tail flush ----------------
            while cursor < 256:
                emit_aV(cursor)
                if (cursor & 15) == 15:
                    emit_drain_norm(cursor >> 6, (cursor >> 4) & 3)
                    if (cursor >> 6) == 3:
                        q3 = (cursor >> 4) & 3
                        for t4 in range(4):
                            for nt in range(2):
                                chainq.append(
                                    [0, deque(out_chain(q3 * 4 + t4, nt)), "Otail"]
                                )
                cursor += 1
            force_finish_due(10**9)
    nc.compile()
    return nc


def _get_nc():
    if "nc" not in _CACHE:
        _CACHE["nc"] = _build_nc()
    return _CACHE["nc"]


def kernel(x, W_qkv, b_qkv, W_out, b_out):
    global LAST_RESULT
    from concourse.bass_utils import run_bass_kernel_spmd

    x = np.asarray(x, dtype=np.float32)
    W_qkv = np.asarray(W_qkv, dtype=np.float32)
    b_qkv = np.asarray(b_qkv, dtype=np.float32)
    W_out = np.asarray(W_out, dtype=np.float32)
    b_out = np.asarray(b_out, dtype=np.float32)

    scale = 1.0 / np.sqrt(HD)
    # [hidden, 3, heads, hd]
    w4 = W_qkv.reshape(HIDDEN, 3, HEADS, HD)
    b4 = b_qkv.reshape(3, HEADS, HD)

    in_maps = []
    for c in range(8):
        b = c // 2
        g = c % 2
        hs = slice(g * HG, (g + 1) * HG)
        wq = (w4[:, 0, hs, :] * scale).reshape(HIDDEN, 512)
        wk = w4[:, 1, hs, :].reshape(HIDDEN, 512)
        wv_ = np.ascontiguousarray(
            w4[:, 2, hs, :].reshape(8, 128, 512).transpose(1, 0, 2)
        ).astype(np.float16)
        # columns of the packed qk weight follow CCX_ORDER so the slices the
        # kernel needs first are contiguous at offset 0
        qk_cols = np.concatenate([wq, wk], axis=1).reshape(HIDDEN, 8, 128)
        qk_cols = qk_cols[:, CCX_ORDER, :].reshape(HIDDEN, 1024)
        wqk = np.ascontiguousarray(
            qk_cols.reshape(8, 128, 1024).transpose(1, 0, 2)
        ).astype(np.float16)
        bq = (b4[0, hs, :] * scale).reshape(512)
        bk = b4[1, hs, :].reshape(512)
        bqk = np.ascontiguousarray(
            np.concatenate([bq, bk]).reshape(8, 128).T
        ).astype(np.float32)
        wout_c = np.ascontiguousarray(
            W_out[g * 512 : (g + 1) * 512, :].reshape(4, 128, HIDDEN).transpose(1, 0, 2)
        ).astype(np.float16)
        xT_b = np.ascontiguousarray(
            x[b].T.reshape(8, 128, SEQ).transpose(1, 0, 2)
        ).astype(np.float16)
        in_maps.append(
            {
                "xT": xT_b,
                "wqk": wqk,
                "wv": wv_,
                "wout": wout_c,
                "bqk": bqk,
            }
        )

    nc = _get_nc()
    res = run_bass_kernel_spmd(
        nc, in_maps, core_ids=list(range(8)), trace=_TRACE
    )
    LAST_RESULT = res

    # host reduction: sum the two head-group partials per batch; fold V-bias
    # and output bias (adding b_v to V shifts every attn output row by b_v,
    # which after the out-projection is the constant b_v @ W_out).
    bv_all = b_qkv[2 * HIDDEN : 3 * HIDDEN]
    const = (b_out + bv_all @ W_out).astype(np.float32)
    out = np.empty((BATCH, SEQ, HIDDEN), dtype=np.float32)
    for b in range(BATCH):
        out[b] = (
            res.results[2 * b]["outp"].astype(np.float32)
            + res.results[2 * b + 1]["outp"].astype(np.float32)
            + const
        )
    return out


# revision 68
# speedup vs baseline: 1.0646x; 1.0646x over previous
"""Multi-head softmax attention (b=4, s=2048, d=1024, 16 heads) on 8 trn2 cores.

Sharding: 2D over (batch, head-half). Core c handles batch c//2, heads
[8*(c%2), 8*(c%2)+8). Each core computes its QKV projections, attention for
its 8 heads, and a partial output projection (row-parallel over its 512
attn-out columns). Host sums the two partials per batch.

Schedule: the kernel is ACT-bound (256 exp activations of [128,1024] at
~1.1us each). Emission is software-pipelined around that stream: scores for
group g+2 are emitted at group g, the exp for group g consumes them, and
attnV lags behind on a deep pp pool. All projection work (V/QK/out) is
chopped into accumulation chains dripped into per-group PE slack by
deadline, so the exp stream starts ~10us in and never starves.
"""

import numpy as np

HIDDEN = 1024
SEQ = 2048
BATCH = 4
HEADS = 16
HG = 8  # heads per core
HD = 64  # head dim

_CACHE = {}
_TRACE = False  # test.py sets this for profiling runs
_DEGEN = False  # debug: baseline-like schedule (no deferral, eager chains)
LAST_RESULT = None

# host-side column-block order of the packed qkv weight: K/Q of pair 0 first
# so the first score groups only need a 256-column DMA slice.
CCX_ORDER = [4, 0, 5, 1, 6, 2, 7, 3]
CCX_OFF = {ccx: i * 128 for i, ccx in enumerate(CCX_ORDER)}


def _build_nc():
    from collections import deque

    import concourse.mybir as mybir
    import concourse.tile as tile
    from concourse import bacc

    f32 = mybir.dt.float32
    f16 = mybir.dt.float16
    Exp = mybir.ActivationFunctionType.Exp

    nc = bacc.Bacc("TRN2", target_bir_lowering=False, debug=False)
    xT = nc.dram_tensor("xT", [128, 8, SEQ], f16, kind="ExternalInput").ap()
    wqk = nc.dram_tensor("wqk", [128, 8, 1024], f16, kind="ExternalInput").ap()
    wv = nc.dram_tensor("wv", [128, 8, 512], f16, kind="ExternalInput").ap()
    wout = nc.dram_tensor("wout", [128, 4, HIDDEN], f16, kind="ExternalInput").ap()
    bqk = nc.dram_tensor("bqk", [128, 8], f32, kind="ExternalInput").ap()
    outp = nc.dram_tensor("outp", [SEQ, HIDDEN], f16, kind="ExternalOutput").ap()

    with tile.TileContext(nc) as tc:
        with (
            tc.tile_pool(name="persist", bufs=1) as pers,
            tc.tile_pool(name="pp", bufs=14) as pppool,
            tc.tile_pool(name="small", bufs=2) as small,
            tc.tile_pool(name="obuf", bufs=3) as obuf,
            tc.tile_pool(name="ps_sc", bufs=2, space="PSUM") as ps_sc,
            tc.tile_pool(name="ps_ac", bufs=1, space="PSUM") as ps_ac,
            tc.tile_pool(name="ps_aux", bufs=2, space="PSUM") as ps_aux,
        ):
            xt16 = pers.tile([128, 8, SEQ], f16, tag="xt16", name="xt16")
            wqk16 = pers.tile([128, 8, 1024], f16, tag="wqk16", name="wqk16")
            wv16 = pers.tile([128, 8, 512], f16, tag="wv16", name="wv16")
            qk = [pers.tile([128, SEQ], f16, tag=f"qk{i}", name=f"qk{i}") for i in range(8)]
            vt = [pers.tile([128, HG, 128], f16, tag=f"vt{i}", name=f"vt{i}") for i in range(16)]
            aot = [pers.tile([128, SEQ], f16, tag=f"aot{i}", name=f"aot{i}") for i in range(4)]
            # fp16 partials for qt3's out-proj pass-1 (pairs 0-1), so only the
            # pairs-2-3 half remains after the final norm
            op3 = [pers.tile([128, 512], f16, tag=f"op3{i}", name=f"op3{i}") for i in range(8)]
            wout_sb = pers.tile([128, 4, HIDDEN], f16, tag="wo", name="wo")
            bqk_sb = pers.tile([128, 8], f32, tag="bqk", name="bqk")
            ones8 = pers.tile([128, HG], f16, tag="ones8", name="ones8")

            # ---------------- DMA emission, need-ordered ----------------
            nc.sync.dma_start(bqk_sb[:], bqk[:])
            for h2 in range(0, 8, 2):  # K0+Q0 slices (host packs them first);
                # 2 hc per transfer: fewer Sync issues, keeps queue parallelism
                nc.sync.dma_start(
                    wqk16[:, h2 : h2 + 2, 0:256], wqk[:, h2 : h2 + 2, 0:256]
                )
            for h2 in range(0, 8, 2):  # xt token-quarter 0
                nc.sync.dma_start(
                    xt16[:, h2 : h2 + 2, 0:512], xT[:, h2 : h2 + 2, 0:512]
                )
            for h2 in range(0, 8, 2):
                nc.sync.dma_start(wv16[:, h2 : h2 + 2, :], wv[:, h2 : h2 + 2, :])
            for t in range(1, 4):
                for h2 in range(0, 8, 2):
                    nc.sync.dma_start(
                        xt16[:, h2 : h2 + 2, t * 512 : (t + 1) * 512],
                        xT[:, h2 : h2 + 2, t * 512 : (t + 1) * 512],
                    )
            for h2 in range(0, 8, 2):  # rest of qkv weights
                nc.sync.dma_start(
                    wqk16[:, h2 : h2 + 2, 256:1024], wqk[:, h2 : h2 + 2, 256:1024]
                )
            for i in range(0, 4, 2):
                nc.sync.dma_start(
                    wout_sb[:, i : i + 2, :], wout[:, i : i + 2, :]
                )
            nc.vector.memset(ones8[:], 1.0)
            # warm the ACT exp table during the DMA lead (table load ~2.7us)
            warm = pers.tile([128, HG], f32, tag="warm", name="warm")
            nc.scalar.activation(warm[:], ones8[:], Exp)
            for t in range(16):
                nc.vector.memset(vt[t][:, :, HD + 1 : 128], 0.0)
                nc.vector.tensor_copy(vt[t][:, :, HD], ones8[:])

            # ---------------- chain builders ----------------
            def aux_psum():
                return ps_aux.tile([128, 512], f32, tag="aux", name="aux")

            qk_done = set()  # (tt, ccx) fully emitted
            vt_done = [False] * 16

            def qk_chain(tt, ccx):
                """Q/K projection chain: qk[ccx][:, tt*512:+512] = W_ccx^T x."""
                cell = {}
                off = CCX_OFF[ccx]
                ops = []

                def mk_mm(hc):
                    def f():
                        if "ps" not in cell:
                            cell["ps"] = aux_psum()
                        nc.tensor.matmul(
                            cell["ps"][:],
                            wqk16[:, hc, off : off + 128],
                            xt16[:, hc, tt * 512 : (tt + 1) * 512],
                            start=(hc == 0),
                            stop=(hc == 7),
                        )
                    return f

                for hc in range(8):
                    ops.append((240, mk_mm(hc)))

                def ev():
                    nc.vector.tensor_scalar_add(
                        qk[ccx][:, tt * 512 : (tt + 1) * 512],
                        cell["ps"][:],
                        bqk_sb[:, ccx : ccx + 1],
                    )
                    qk_done.add((tt, ccx))
                return ops + [(0, ev)]

            def v_chain(t):
                """V projection chain for token chunk t (token-major + ones col)."""
                cell = {}
                ops = []

                def mk_mm(hc):
                    def f():
                        if "ps" not in cell:
                            cell["ps"] = aux_psum()
                        nc.tensor.matmul(
                            cell["ps"][:],
                            xt16[:, hc, t * 128 : (t + 1) * 128],
                            wv16[:, hc, :],
                            start=(hc == 0),
                            stop=(hc == 7),
                        )
                    return f

                for hc in range(8):
                    ops.append((240, mk_mm(hc)))

                def ev():
                    nc.vector.tensor_copy(
                        vt[t][:, :, 0:HD],
                        cell["ps"][:].rearrange("p (h d) -> p h d", h=HG),
                    )
                    vt_done[t] = True
                return ops + [(0, ev)]

            def out_chain(tch, nt):
                """Output projection chain for (token chunk, 512-col half)."""
                cell = {}
                ops = []

                def mk_mm(pair_):
                    def f():
                        if "ps" not in cell:
                            cell["ps"] = aux_psum()
                        nc.tensor.matmul(
                            cell["ps"][:],
                            aot[pair_][:, tch * 128 : (tch + 1) * 128],
                            wout_sb[:, pair_, nt * 512 : (nt + 1) * 512],
                            start=(pair_ == 0),
                            stop=(pair_ == 3),
                        )
                    return f

                for pair_ in range(4):
                    ops.append((240, mk_mm(pair_)))

                def ev():
                    ot = obuf.tile([128, 512], f16, tag="ot", name="ot")
                    nc.vector.tensor_copy(ot[:], cell["ps"][:])
                    nc.sync.dma_start(
                        outp[tch * 128 : (tch + 1) * 128, nt * 512 : (nt + 1) * 512],
                        ot[:],
                    )
                return ops + [(0, ev)]

            def out_p3(idx, tch, nt, phase):
                """qt3 out-proj in two passes: pairs 0-1 to fp16 partials
                (phase 1, drippable early), pairs 2-3 + add-evict (phase 2)."""
                cell = {}
                ops = []
                p0 = 0 if phase == 1 else 2

                def mk_mm(pair_):
                    def f():
                        if "ps" not in cell:
                            cell["ps"] = aux_psum()
                        nc.tensor.matmul(
                            cell["ps"][:],
                            aot[pair_][:, tch * 128 : (tch + 1) * 128],
                            wout_sb[:, pair_, nt * 512 : (nt + 1) * 512],
                            start=(pair_ == p0),
                            stop=(pair_ == p0 + 1),
                        )
                    return f

                for pair_ in (p0, p0 + 1):
                    ops.append((240, mk_mm(pair_)))

                def ev():
                    if phase == 1:
                        nc.vector.tensor_copy(op3[idx][:], cell["ps"][:])
                    else:
                        ot = obuf.tile([128, 512], f16, tag="ot", name="ot")
                        nc.vector.tensor_add(ot[:], cell["ps"][:], op3[idx][:])
                        nc.sync.dma_start(
                            outp[
                                tch * 128 : (tch + 1) * 128,
                                nt * 512 : (nt + 1) * 512,
                            ],
                            ot[:],
                        )
                return ops + [(0, ev)]

            # ---------------- group primitives ----------------
            # group g: pair = g>>6, qt = (g>>4)&3, kc = g&15
            sc_of = {}
            pp_of = {}
            acc_cur = {}

            def emit_S(g):
                pair, qt, kc = g >> 6, (g >> 4) & 3, g & 15
                assert (kc // 4, 4 + pair) in qk_done, f"K chain missing for g={g}"
                assert (qt, pair) in qk_done, f"Q chain missing for g={g}"
                qtile = qk[pair]
                ktile = qk[4 + pair]
                sc = ps_sc.tile([128, 1024], f32, tag="sc", name="sc")
                sc_of[g] = sc
                nc.tensor.matmul(
                    sc[:, 0:512],
                    ktile[0:64, kc * 128 : (kc + 1) * 128],
                    qtile[0:64, qt * 512 : (qt + 1) * 512],
                    start=True,
                    stop=True,
                    tile_position=(0, 0),
                )
                nc.tensor.matmul(
                    sc[:, 512:1024],
                    ktile[64:128, kc * 128 : (kc + 1) * 128],
                    qtile[64:128, qt * 512 : (qt + 1) * 512],
                    start=True,
                    stop=True,
                    tile_position=(64, 0),
                )

            def emit_exp(g):
                pp = pppool.tile([128, 1024], f16, tag="pp", name="pp")
                pp_of[g] = pp
                nc.scalar.activation(pp[:], sc_of.pop(g)[:], Exp)

            def emit_aV(g):
                pair, qt, kc = g >> 6, (g >> 4) & 3, g & 15
                assert vt_done[kc], f"vt missing for g={g}"
                if kc == 0:
                    acc_cur["A"] = ps_ac.tile([128, 512], f32, tag="accA", name="accA")
                    acc_cur["B"] = ps_ac.tile([128, 512], f32, tag="accB", name="accB")
                pp = pp_of.pop(g)
                nc.tensor.matmul(
                    acc_cur["A"][:],
                    vt[kc][:, 2 * pair, :],
                    pp[:, 0:512],
                    start=(kc == 0),
                    stop=(kc == 15),
                )
                nc.tensor.matmul(
                    acc_cur["B"][:],
                    vt[kc][:, 2 * pair + 1, :],
                    pp[:, 512:1024],
                    start=(kc == 0),
                    stop=(kc == 15),
                )

            def emit_drain_norm(pair, qt):
                # drain PSUM accumulators promptly so the banks free fast
                parts = []
                for key, row0 in (("A", 0), ("B", 64)):
                    acc = acc_cur[key]
                    dn = small.tile([1, 512], f32, tag="dn", name="dn")
                    nc.vector.tensor_copy(dn[:], acc[64:65, :])
                    numer = small.tile([64, 512], f32, tag="numer", name="numer")
                    nc.vector.tensor_copy(numer[:], acc[0:64, :])
                    parts.append((dn, numer, row0))
                for dn, numer, row0 in parts:
                    bc = small.tile([64, 512], f32, tag="bc", name="bc")
                    nc.gpsimd.partition_broadcast(bc[:], dn[:])
                    rc = small.tile([64, 512], f32, tag="rc", name="rc")
                    scr = small.tile([64, 512], f32, tag="scr", name="scr")
                    nc.vector.reciprocal_approx_accurate(rc[:], bc[:], scr[:])
                    nc.vector.tensor_mul(
                        aot[pair][row0 : row0 + 64, qt * 512 : (qt + 1) * 512],
                        numer[:],
                        rc[:],
                    )

            # ---------------- lead ----------------
            lead_k = qk_chain(0, 4)
            lead_q = qk_chain(0, 0)
            for _, f in lead_k:
                f()
            for _, f in lead_q:
                f()
            emit_S(0)
            emit_S(1)

            # ---------------- chain queue with deadlines ----------------
            chainq = []  # sorted by due: [due, deque(ops), label]
            for t in range(16):  # V projection upfront: a clean serial lead
                for _, f in v_chain(t):
                    f()
            for tt in range(1, 4):
                chainq.append([4 * tt - 3, deque(qk_chain(tt, 4)), f"K0t{tt}"])
            for tt in range(1, 4):
                chainq.append([16 * tt - 3, deque(qk_chain(tt, 0)), f"Q0t{tt}"])
            for p in range(1, 4):
                for tt in range(4):
                    chainq.append(
                        [64 * p + 4 * tt - 7, deque(qk_chain(tt, 4 + p)), f"K{p}t{tt}"]
                    )
                for tt in range(4):
                    chainq.append(
                        [64 * p + 16 * tt - 9, deque(qk_chain(tt, p)), f"Q{p}t{tt}"]
                    )
            chainq.sort(key=lambda c: c[0])

            # ---------------- main group loop ----------------
            cursor = 0  # next aV unit to emit
            partial = []  # front chain being dripped; never preempted

            def fill_ops(budget):
                # emit chain ops one at a time; a partially-emitted chain owns
                # an aux-psum slot, so it must finish before any other starts
                while True:
                    if not partial:
                        if not chainq:
                            return budget
                        if budget <= 150:
                            return budget
                        partial.append(chainq.pop(0))
                    c = partial[0]
                    while c[1] and budget > 150:
                        cost, f = c[1].popleft()
                        f()
                        budget -= max(cost, 60)
                    if c[1]:
                        return budget
                    partial.pop()

            def force_finish_due(g):
                # complete the partial chain, then any chain whose due passed
                while partial and partial[0][0] <= g:
                    c = partial.pop()
                    while c[1]:
                        c[1].popleft()[1]()
                while chainq and chainq[0][0] <= g:
                    if partial:  # a later-due chain is mid-flight: finish it
                        c = partial.pop()
                    else:
                        c = chainq.pop(0)
                    while c[1]:
                        c[1].popleft()[1]()

            def caps(g):
                if _DEGEN:
                    return 10**9
                if g < 48:
                    return 1250
                if g >= 192:
                    return 1500
                return 1160

            for g in range(256):
                emit_exp(g)
                # S goes first: it is the only PE work on the exp stream's
                # critical path (exp(g+2) needs it); chain dues guarantee its
                # qk inputs were emitted in earlier groups
                if g + 2 < 256:
                    emit_S(g + 2)
                force_finish_due(g)
                budget = caps(g) - 330
                while cursor <= g and vt_done[cursor & 15] and (
                    budget > 0 or (g - cursor) >= 11
                ):
                    emit_aV(cursor)
                    budget -= 440
                    if (cursor & 15) == 15:
                        emit_drain_norm(cursor >> 6, (cursor >> 4) & 3)
                        p3, q3 = cursor >> 6, (cursor >> 4) & 3
                        if p3 == 1 and q3 == 3:
                            # aot[0..1] complete for qt3: queue its out-proj
                            # pass-1 (pairs 0-1 -> fp16 partials)
                            for i, (t4, nt) in enumerate(
                                (t4, nt) for t4 in range(4) for nt in range(2)
                            ):
                                chainq.append(
                                    [g + 12 + 4 * i,
                                     deque(out_p3(i, 12 + t4, nt, 1)),
                                     f"P1{t4}{nt}"]
                                )
                            chainq.sort(key=lambda c: c[0])
                        if p3 == 3:  # out-proj cells become available
                            for i, (t4, nt) in enumerate(
                                (t4, nt) for t4 in range(4) for nt in range(2)
                            ):
                                ch = (
                                    out_p3(i, 12 + t4, nt, 2)
                                    if q3 == 3
                                    else out_chain(q3 * 4 + t4, nt)
                                )
                                chainq.append(
                                    [g + 10 + 2 * i, deque(ch), f"O{q3}{t4}{nt}"]
                                )
                            chainq.sort(key=lambda c: c[0])
                    cursor += 1
                fill_ops(budget)

            # ---------------- tail flush ----------------
            while cursor < 256:
                emit_aV(cursor)
                if (cursor & 15) == 15:
                    emit_drain_norm(cursor >> 6, (cursor >> 4) & 3)
                    if (cursor >> 6) == 3:
                        q3 = (cursor >> 4) & 3
                        for t4 in range(4):
                            for nt in range(2):
                                ch = (
                                    out_p3(t4 * 2 + nt, 12 + t4, nt, 2)
                                    if q3 == 3
                                    else out_chain(q3 * 4 + t4, nt)
                                )
                                chainq.append([0, deque(ch), "Otail"])
                cursor += 1
            force_finish_due(10**9)
    nc.compile()
    return nc


def _get_nc():
    if "nc" not in _CACHE:
        _CACHE["nc"] = _build_nc()
    return _CACHE["nc"]


def kernel(x, W_qkv, b_qkv, W_out, b_out):
    global LAST_RESULT
    from concourse.bass_utils import run_bass_kernel_spmd

    x = np.asarray(x, dtype=np.float32)
    W_qkv = np.asarray(W_qkv, dtype=np.float32)
    b_qkv = np.asarray(b_qkv, dtype=np.float32)
    W_out = np.asarray(W_out, dtype=np.float32)
    b_out = np.asarray(b_out, dtype=np.float32)

    scale = 1.0 / np.sqrt(HD)
    # [hidden, 3, heads, hd]
    w4 = W_qkv.reshape(HIDDEN, 3, HEADS, HD)
    b4 = b_qkv.reshape(3, HEADS, HD)

    in_maps = []
    for c in range(8):
        b = c // 2
        g = c % 2
        hs = slice(g * HG, (g + 1) * HG)
        wq = (w4[:, 0, hs, :] * scale).reshape(HIDDEN, 512)
        wk = w4[:, 1, hs, :].reshape(HIDDEN, 512)
        wv_ = np.ascontiguousarray(
            w4[:, 2, hs, :].reshape(8, 128, 512).transpose(1, 0, 2)
        ).astype(np.float16)
        # columns of the packed qk weight follow CCX_ORDER so the slices the
        # kernel needs first are contiguous at offset 0
        qk_cols = np.concatenate([wq, wk], axis=1).reshape(HIDDEN, 8, 128)
        qk_cols = qk_cols[:, CCX_ORDER, :].reshape(HIDDEN, 1024)
        wqk = np.ascontiguousarray(
            qk_cols.reshape(8, 128, 1024).transpose(1, 0, 2)
        ).astype(np.float16)
        bq = (b4[0, hs, :] * scale).reshape(512)
        bk = b4[1, hs, :].reshape(512)
        bqk = np.ascontiguousarray(
            np.concatenate([bq, bk]).reshape(8, 128).T
        ).astype(np.float32)
        wout_c = np.ascontiguousarray(
            W_out[g * 512 : (g + 1) * 512, :].reshape(4, 128, HIDDEN).transpose(1, 0, 2)
        ).astype(np.float16)
        xT_b = np.ascontiguousarray(
            x[b].T.reshape(8, 128, SEQ).transpose(1, 0, 2)
        ).astype(np.float16)
        in_maps.append(
            {
                "xT": xT_b,
                "wqk": wqk,
                "wv": wv_,
                "wout": wout_c,
                "bqk": bqk,
            }
        )

    nc = _get_nc()
    res = run_bass_kernel_spmd(
        nc, in_maps, core_ids=list(range(8)), trace=_TRACE
    )
    LAST_RESULT = res

    # host reduction: sum the two head-group partials per batch; fold V-bias
    # and output bias (adding b_v to V shifts every attn output row by b_v,
    # which after the out-projection is the constant b_v @ W_out).
    bv_all = b_qkv[2 * HIDDEN : 3 * HIDDEN]
    const = (b_out + bv_all @ W_out).astype(np.float32)
    out = np.empty((BATCH, SEQ, HIDDEN), dtype=np.float32)
    for b in range(BATCH):
        out[b] = (
            res.results[2 * b]["outp"].astype(np.float32)
            + res.results[2 * b + 1]["outp"].astype(np.float32)
            + const
        )
    return out
